# revision 1
# baseline (speedup 1.0000x reference)
"""DiffuseRouter kernel for 8 TRN2 NeuronCores.

Reference computation (enable_time=False, soft_time_routing=True):
    out[b, l, d] = (1/3) * sum_g sum_e expert_emb_g[e, b, l, d]
i.e. a uniform-weighted sum of 28 expert planes per batch element.

Sharding: pure data-parallel over batch B=8 -> one batch element per core.
Each core reads its 28 [256, 1280] f32 planes (36.7 MB), reduces them
on-chip, scales by 1/3, and writes its [256, 1280] output.  No collectives
needed (B == n_cores), which is strictly less traffic than expert-parallel
+ all-reduce.
"""

import numpy as np

import concourse.bacc as bacc
import concourse.tile as tile
from concourse import mybir
from concourse.alu_op_type import AluOpType
from concourse.bass_utils import run_bass_kernel_spmd

N_CORES = 8
E_TOTAL = 28  # 4 + 8 + 16 experts across the 3 granularity levels
L, D = 256, 1280
P = 128  # SBUF partitions
FD = (L // P) * D  # 2560 free-dim elements per partition
SCALE = 1.0 / 3.0

_NC_CACHE = None


def _build_nc():
    """Build the SPMD Bass program (identical on all 8 cores).

    Structure: stream the 28 expert planes as [128, 2560] tiles (1.31 MB
    linear DMAs) on the SP HWDGE ring; accumulate on DVE in two independent
    half-chains over the free dim (cols [0:1280) and [1280:2560)) with the
    1/3 scale folded into every add via scalar_tensor_tensor, so each half
    can be stored the moment its last add retires.  Stores go on the ACT
    HWDGE ring so they never queue behind input loads.
    """
    nc = bacc.Bacc(
        "TRN2", target_bir_lowering=False, debug=False, enable_partition_id=False
    )
    x = nc.dram_tensor("x", [E_TOTAL, L, D], mybir.dt.float32, kind="ExternalInput")
    out = nc.dram_tensor("out", [L, D], mybir.dt.float32, kind="ExternalOutput")

    # [E, 256, 1280] -> [E, 128, 2560]: partition p holds rows 2p, 2p+1
    # (contiguous 10240 B per partition -> fully linear 1.31 MB DMA per plane).
    x_t = x.ap().rearrange("e (p a) d -> e p (a d)", a=2)
    out_t = out.ap().rearrange("(p a) d -> p (a d)", a=2)

    H = FD // 2  # half of the free dim
    halves = [slice(0, H), slice(H, FD)]
    mult = AluOpType.mult
    add = AluOpType.add

    with tile.TileContext(nc) as tc:
        with (
            tc.tile_pool(name="in", bufs=8) as pin,
            tc.tile_pool(name="acc", bufs=2) as pacc,
        ):
            accs = [
                pacc.tile([P, H], mybir.dt.float32, name=f"acc{i}", tag=f"acc{i}")
                for i in range(2)
            ]
            last = E_TOTAL - 1
            for e in range(E_TOTAL):
                if e < last:
                    # All input loads on the SP HWDGE ring: strict FIFO order
                    # matches the accumulation order, so exactly one tile's
                    # adds remain after the stream ends.
                    t = pin.tile([P, FD], mybir.dt.float32)
                    nc.sync.dma_start(out=t[:], in_=x_t[e])
                    ths = [t[:, h] for h in halves]
                else:
                    # Last expert: four quarter-loads in separate tiles so
                    # each final quarter-add starts as soon as its own
                    # quarter lands (not its half).
                    Q = FD // 4
                    qts = []
                    for qi in range(4):
                        qt = pin.tile(
                            [P, Q], mybir.dt.float32, name=f"tq{qi}", tag=f"tq{qi}"
                        )
                        nc.sync.dma_start(
                            out=qt[:], in_=x_t[e][:, qi * Q : (qi + 1) * Q]
                        )
                        qts.append(qt[:])
                    ths = qts
                if e < last:
                    for acc, th in zip(accs, ths):
                        if e == 0:
                            # acc = t0 * 1/3 (tensor_scalar: 2x perf mode)
                            nc.vector.tensor_scalar_mul(acc[:], th, SCALE)
                        else:
                            # acc = (t_e * 1/3) + acc
                            nc.vector.scalar_tensor_tensor(
                                acc[:], th, SCALE, acc[:], mult, add
                            )
                else:
                    # Final adds split into quarters so each quarter-store
                    # can fire as soon as its own quarter retires.
                    Q = H // 2
                    for qi in range(4):
                        acc = accs[qi // 2]
                        q = slice((qi % 2) * Q, (qi % 2 + 1) * Q)
                        nc.vector.scalar_tensor_tensor(
                            acc[:, q], ths[qi], SCALE, acc[:, q], mult, add
                        )
            # Quarter-stores alternating rings per quarter (ACT, SP, ACT, SP)
            # so consecutive quarters never queue behind each other in one
            # ring's FIFO — the last quarter's store issues immediately.
            Q = H // 2
            for hi, acc in enumerate(accs):
                for qi in range(2):
                    q = slice(qi * Q, (qi + 1) * Q)
                    gq = slice(hi * H + qi * Q, hi * H + (qi + 1) * Q)
                    eng = nc.scalar if (hi * 2 + qi) % 2 == 0 else nc.sync
                    eng.dma_start(out=out_t[:, gq], in_=acc[:, q])
    nc.compile()
    return nc


def _get_nc():
    global _NC_CACHE
    if _NC_CACHE is None:
        _NC_CACHE = _build_nc()
    return _NC_CACHE


def _run(inputs, trace=False, trace_kwargs=None):
    e0 = np.asarray(inputs["expert_emb_0"], dtype=np.float32)
    e1 = np.asarray(inputs["expert_emb_1"], dtype=np.float32)
    e2 = np.asarray(inputs["expert_emb_2"], dtype=np.float32)
    B = e0.shape[1]
    assert B == N_CORES, f"expected B == {N_CORES}, got {B}"

    in_maps = []
    for b in range(B):
        xb = np.concatenate([e0[:, b], e1[:, b], e2[:, b]], axis=0)
        in_maps.append({"x": np.ascontiguousarray(xb)})

    kw = {}
    if trace:
        kw["trace"] = True
        if trace_kwargs:
            kw.update(trace_kwargs)
    try:
        res = run_bass_kernel_spmd(_get_nc(), in_maps, list(range(N_CORES)), **kw)
    except Exception:
        # One retry: transient device errors (e.g. NRT unrecoverable after a
        # prior wedged run) usually clear on re-dispatch.
        res = run_bass_kernel_spmd(_get_nc(), in_maps, list(range(N_CORES)), **kw)
    out = np.stack([res.results[b]["out"] for b in range(B)], axis=0)
    return out.astype(np.float32, copy=False), res


def kernel(**inputs) -> np.ndarray:
    out, _ = _run(inputs, trace=False)
    return out



# revision 7
# speedup vs baseline: 1.0911x; 1.0911x over previous
"""DiffuseRouter kernel for 8 TRN2 NeuronCores.

Reference computation (enable_time=False, soft_time_routing=True):
    out[b, l, d] = (1/3) * sum_g sum_e expert_emb_g[e, b, l, d]
i.e. a uniform-weighted sum of 28 expert planes per batch element.

Sharding: pure data-parallel over batch B=8 -> one batch element per core.
Each core streams its 28 [256, 1280] f32 planes (36.7 MB) over the SP HWDGE
ring at ~417 GB/s and reduces them on the fly.  The reduction is split by
free-dim columns across two engines so neither can fall behind the DMA
stream (the previous all-DVE version lagged ~11 us):

  - cols [0:W_V):    DVE accumulator chain (fp32 tensor_tensor adds)
  - cols [W_V:2560): TensorE identity-matmul accumulation into PSUM banks
                     (fp32r moving operand -> 1 cycle/row), evacuated at the
                     end by ScalarE with the 1/3 scale folded in.

The last plane is loaded in per-stripe pieces so each engine's final op and
the output stores fire progressively as pieces land.
"""

import numpy as np

import concourse.bacc as bacc
import concourse.tile as tile
from concourse import mybir
from concourse.alu_op_type import AluOpType
from concourse.bass_utils import run_bass_kernel_spmd

N_CORES = 8
E_TOTAL = 28  # 4 + 8 + 16 experts across the 3 granularity levels
L, D = 256, 1280
P = 128  # SBUF partitions
FD = (L // P) * D  # 2560 free-dim elements per partition
SCALE = 1.0 / 3.0

BANK = 512  # one PSUM bank = 512 fp32
W_V = 1536  # DVE stripe width (cols [0:W_V))
NB = (FD - W_V) // BANK  # PSUM banks for the PE stripe
PE_F32R = True  # fp32r moving operand (1 cyc/row) vs fp32 (4 cyc/row)

_NC_CACHE = None


def _build_nc():
    nc = bacc.Bacc(
        "TRN2", target_bir_lowering=False, debug=False, enable_partition_id=False
    )
    x = nc.dram_tensor("x", [E_TOTAL, L, D], mybir.dt.float32, kind="ExternalInput")
    ident = nc.dram_tensor("ident", [P, P], mybir.dt.float32, kind="ExternalInput")
    out = nc.dram_tensor("out", [L, D], mybir.dt.float32, kind="ExternalOutput")

    # [E, 256, 1280] -> [E, 128, 2560]: partition p holds rows 2p, 2p+1
    # (contiguous 10240 B per partition -> fully linear 1.31 MB DMA per plane).
    x_t = x.ap().rearrange("e (p a) d -> e p (a d)", a=2)
    out_t = out.ap().rearrange("(p a) d -> p (a d)", a=2)

    f32 = mybir.dt.float32
    f32r = mybir.dt.float32r
    mult, add = AluOpType.mult, AluOpType.add
    last = E_TOTAL - 1
    n_vchunk = W_V // BANK

    # walrus requires fp32r-matmul operands to be *produced* as fp32r, so the
    # plane tiles are declared fp32r (the DMA is a pure 4-byte copy either
    # way) and the DVE reads its stripe bitcast back to fp32.
    pe_dt = f32r if PE_F32R else f32

    def as_dve(ap):
        return ap.bitcast(f32) if PE_F32R else ap

    def dma_src(ap):
        return ap.bitcast(f32r) if PE_F32R else ap

    with tile.TileContext(nc) as tc:
        with (
            tc.tile_pool(name="in", bufs=8) as pin,
            tc.tile_pool(name="pieces", bufs=1) as ppiece,
            tc.tile_pool(name="single", bufs=1) as psingle,
            tc.tile_pool(name="psum", bufs=1, space="PSUM") as ppsum,
        ):
            ident_sb = psingle.tile([P, P], pe_dt, name="ident", tag="ident")
            acc = psingle.tile([P, W_V], f32, name="acc", tag="acc")
            stage = [
                psingle.tile([P, BANK], f32, name=f"stage{b}", tag=f"stage{b}")
                for b in range(NB)
            ]
            banks = [
                ppsum.tile([P, BANK], f32, name=f"bank{b}", tag=f"bank{b}")
                for b in range(NB)
            ]

            # Identity for the PE accumulation; ACT ring so it never queues
            # ahead of plane loads on the SP ring.
            nc.scalar.dma_start(out=ident_sb[:], in_=dma_src(ident.ap()))

            for e in range(last):
                t = pin.tile([P, FD], pe_dt)
                nc.sync.dma_start(out=t[:], in_=dma_src(x_t[e]))
                # DVE stripe
                if e == 0:
                    nc.vector.tensor_copy(acc[:], as_dve(t[:, 0:W_V]))
                else:
                    nc.vector.tensor_add(acc[:], acc[:], as_dve(t[:, 0:W_V]))
                # PE stripe: accumulate into PSUM banks
                for b in range(NB):
                    c0 = W_V + b * BANK
                    nc.tensor.matmul(
                        banks[b][:],
                        ident_sb[:],
                        t[:, c0 : c0 + BANK],
                        start=(e == 0),
                        stop=False,
                    )

            # Pre-scale the DVE accumulator while plane 27 is in flight.
            nc.vector.tensor_scalar_mul(acc[:], acc[:], SCALE)

            # Last plane in pieces: PE banks first (their finish chain is
            # longer: MM -> ACT evac -> store), then the DVE chunks.
            pe_pieces = []
            for b in range(NB):
                c0 = W_V + b * BANK
                q = ppiece.tile([P, BANK], pe_dt, name=f"pq{b}", tag=f"pq{b}")
                nc.sync.dma_start(
                    out=q[:], in_=dma_src(x_t[last][:, c0 : c0 + BANK])
                )
                pe_pieces.append(q)
            dv_pieces = []
            for c in range(n_vchunk):
                q = ppiece.tile([P, BANK], f32, name=f"dq{c}", tag=f"dq{c}")
                nc.sync.dma_start(
                    out=q[:], in_=x_t[last][:, c * BANK : (c + 1) * BANK]
                )
                dv_pieces.append(q)

            # PE: close each accumulation group, evacuate with scale, store.
            for b in range(NB):
                c0 = W_V + b * BANK
                nc.tensor.matmul(
                    banks[b][:],
                    ident_sb[:],
                    pe_pieces[b][:],
                    start=False,
                    stop=True,
                )
                nc.scalar.mul(stage[b][:], banks[b][:], SCALE)
                nc.scalar.dma_start(out=out_t[:, c0 : c0 + BANK], in_=stage[b][:])

            # DVE: final fused (x*1/3 + scaled_acc) per chunk, store per chunk.
            for c in range(n_vchunk):
                sl = slice(c * BANK, (c + 1) * BANK)
                nc.vector.scalar_tensor_tensor(
                    acc[:, sl], dv_pieces[c][:], SCALE, acc[:, sl], mult, add
                )
                nc.sync.dma_start(out=out_t[:, sl], in_=acc[:, sl])
    nc.compile()
    return nc


def _get_nc():
    global _NC_CACHE
    if _NC_CACHE is None:
        _NC_CACHE = _build_nc()
    return _NC_CACHE


_IDENT = np.eye(P, dtype=np.float32)


def _run(inputs, trace=False, trace_kwargs=None):
    e0 = np.asarray(inputs["expert_emb_0"], dtype=np.float32)
    e1 = np.asarray(inputs["expert_emb_1"], dtype=np.float32)
    e2 = np.asarray(inputs["expert_emb_2"], dtype=np.float32)
    B = e0.shape[1]
    assert B == N_CORES, f"expected B == {N_CORES}, got {B}"

    in_maps = []
    for b in range(B):
        xb = np.concatenate([e0[:, b], e1[:, b], e2[:, b]], axis=0)
        in_maps.append({"x": np.ascontiguousarray(xb), "ident": _IDENT})

    kw = {}
    if trace:
        kw["trace"] = True
        if trace_kwargs:
            kw.update(trace_kwargs)
    try:
        res = run_bass_kernel_spmd(_get_nc(), in_maps, list(range(N_CORES)), **kw)
    except Exception:
        # One retry: transient device errors (e.g. NRT unrecoverable after a
        # prior wedged run) usually clear on re-dispatch.
        res = run_bass_kernel_spmd(_get_nc(), in_maps, list(range(N_CORES)), **kw)
    out = np.stack([res.results[b]["out"] for b in range(B)], axis=0)
    return out.astype(np.float32, copy=False), res


def kernel(**inputs) -> np.ndarray:
    out, _ = _run(inputs, trace=False)
    return out
